# revision 6
# baseline (speedup 1.0000x reference)
"""Multi-head attention (B=2, S=2048, E=1024, H=16) on 8 TRN2 NeuronCores.

Sharding: core c -> batch b = c//4, head-group g = c%4 (4 heads each).
Megatron-style: col-sharded Wq/Wk/Wv, row-sharded Wo; host sums the 4
partial outputs per batch (the "all-reduce") and adds the bias terms.

Device kernel (per core), all bf16 matmuls with fp32 PSUM accumulate:
  qT/kT  [e=256, s] = W.T-slice @ x.T     (x passed pre-transposed)
  v_aug  [s, 4*65]  = x.T proj, + ones col per head (rowsum trick)
  scoresT[sk, sq]   = kT.T @ qT           (two heads packed in PE rows)
  e = exp(scoresT/8)(*mask)  ->  vals_psum[65, sq] += v_aug.T @ e
  row 64 of vals_psum = softmax denominator; reciprocal + broadcast
  valsT  [e, s] = vals/denominator ; partial_out = valsT.T @ WoT-slice
"""

import numpy as np
import ml_dtypes

B, S, E, H = 2, 2048, 1024, 16
D = E // H            # 64 head dim
N_CORES = 8
GPB = 4               # head-groups per batch
HC = H // GPB         # 4 heads per core
EH = HC * D           # 256 channels per core
PAIRS = HC // 2       # 2 head-pairs per core
P = 128               # partitions
SQ = 512              # sq tile (moving free dim)
KC = E // P           # 8 contraction chunks for projections
NS4 = S // SQ         # 4 sq chunks
NS16 = S // P         # 16

BF16 = ml_dtypes.bfloat16

_cache = {}


def _build(causal: bool, has_bq: bool, has_bk: bool):
    import concourse.bacc as bacc
    import concourse.mybir as mybir
    from concourse.tile import TileContext

    bf16 = mybir.dt.bfloat16
    f32 = mybir.dt.float32
    f32r = mybir.dt.float32r
    EXP = mybir.ActivationFunctionType.Exp

    nc = bacc.Bacc("TRN2", target_bir_lowering=False, debug=False,
                   num_devices=N_CORES)

    xq_d = nc.dram_tensor("xq", [E, S], bf16, kind="ExternalInput")
    xk_d = nc.dram_tensor("xk", [E, S], bf16, kind="ExternalInput")
    xv_d = nc.dram_tensor("xv", [E, S], bf16, kind="ExternalInput")
    wq_d = nc.dram_tensor("wq", [E, EH], bf16, kind="ExternalInput")
    wk_d = nc.dram_tensor("wk", [E, EH], bf16, kind="ExternalInput")
    wv_d = nc.dram_tensor("wv", [E, EH], bf16, kind="ExternalInput")
    wo_d = nc.dram_tensor("wo", [EH, E], bf16, kind="ExternalInput")
    if causal:
        cm_d = nc.dram_tensor("cmask", [4 * P, SQ], bf16, kind="ExternalInput")
    else:
        mt_d = nc.dram_tensor("maskT", [S, S], bf16, kind="ExternalInput")
    if has_bq:
        bq_d = nc.dram_tensor("bq", [1, EH], bf16, kind="ExternalInput")
    if has_bk:
        bk_d = nc.dram_tensor("bk", [1, EH], bf16, kind="ExternalInput")
    out_d = nc.dram_tensor("outp", [S, E], f32, kind="ExternalOutput")

    with TileContext(nc) as tc:
        with (
            tc.tile_pool(name="wpool", bufs=1) as wpool,
            tc.tile_pool(name="xq_pool", bufs=KC) as xq_pool,
            tc.tile_pool(name="xk_pool", bufs=KC) as xk_pool,
            tc.tile_pool(name="xv_pool", bufs=KC) as xv_pool,
            tc.tile_pool(name="qkv", bufs=1) as qkv,
            tc.tile_pool(name="vaug", bufs=NS16) as vaug_pool,
            tc.tile_pool(name="expp", bufs=6) as expp,
            tc.tile_pool(name="small", bufs=4) as small,
            tc.tile_pool(name="mtp", bufs=4) as mtp,
            tc.tile_pool(name="psS", bufs=4, space="PSUM") as psS,
            tc.tile_pool(name="psV", bufs=2, space="PSUM") as psV,
            tc.tile_pool(name="psO", bufs=2, space="PSUM") as psO,
        ):
            # ---- load weights ----
            w_sb = {}
            for nm, d in (("wq", wq_d), ("wk", wk_d), ("wv", wv_d)):
                t = wpool.tile([P, KC * EH], bf16, name=f"{nm}_sb")
                nc.sync.dma_start(
                    out=t[:].rearrange("p (kc e) -> p kc e", e=EH),
                    in_=d.ap().rearrange("(kc p) e -> p kc e", p=P),
                )
                w_sb[nm] = t
            wo_sb = wpool.tile([P, 2 * E], bf16, name="wo_sb")
            nc.sync.dma_start(
                out=wo_sb[:].rearrange("p (ec f) -> p ec f", f=E),
                in_=wo_d.ap().rearrange("(ec p) f -> p ec f", p=P),
            )
            if causal:
                cm_sb = wpool.tile([P, 4 * SQ], bf16, name="cm_sb")
                nc.sync.dma_start(
                    out=cm_sb[:].rearrange("p (j s) -> p j s", s=SQ),
                    in_=cm_d.ap().rearrange("(j p) s -> p j s", p=P),
                )
            if has_bq:
                bq_sb = wpool.tile([1, EH], bf16, name="bq_sb")
                nc.sync.dma_start(out=bq_sb[:], in_=bq_d.ap())
            if has_bk:
                bk_sb = wpool.tile([1, EH], bf16, name="bk_sb")
                nc.sync.dma_start(out=bk_sb[:], in_=bk_d.ap())
            if has_bq or has_bk:
                ones_sq = wpool.tile([1, SQ], bf16, name="ones_sq")
                nc.vector.memset(ones_sq[:], 1.0)

            # ---- load x chunks ----
            def load_x(pool, d, nm):
                view = d.ap().rearrange("(kc p) s -> kc p s", p=P)
                tiles = []
                for kc in range(KC):
                    t = pool.tile([P, S], bf16, name=f"{nm}{kc}", tag=nm)
                    nc.sync.dma_start(out=t[:], in_=view[kc])
                    tiles.append(t)
                return tiles

            xv_sb = load_x(xv_pool, xv_d, "xv")
            xq_sb = load_x(xq_pool, xq_d, "xq")
            xk_sb = load_x(xk_pool, xk_d, "xk")

            # ---- V projection -> v_aug tiles [128, 4*65] ----
            v_aug = []
            for s16 in range(NS16):
                ps = psO.tile([P, EH], f32, tag="psO")
                for kc in range(KC):
                    nc.tensor.matmul(
                        ps[:],
                        lhsT=xv_sb[kc][:, s16 * P:(s16 + 1) * P],
                        rhs=w_sb["wv"][:, kc * EH:(kc + 1) * EH],
                        start=(kc == 0), stop=(kc == KC - 1),
                    )
                va = vaug_pool.tile([P, HC * (D + 1)], bf16,
                                    name=f"vaug{s16}", tag="vaug")
                nc.vector.tensor_copy(
                    out=va[:].rearrange("p (h z) -> p h z", z=D + 1)[:, :, 0:D],
                    in_=ps[:].rearrange("p (h d) -> p h d", d=D),
                )
                nc.vector.memset(
                    va[:].rearrange("p (h z) -> p h z", z=D + 1)[:, :, D:D + 1],
                    1.0,
                )
                v_aug.append(va)

            # ---- Q/K projections for pair c -> qT/kT [128, S] bf16 ----
            qT = [qkv.tile([P, S], bf16, name=f"qT{c}") for c in range(PAIRS)]
            kT = [qkv.tile([P, S], bf16, name=f"kT{c}") for c in range(PAIRS)]
            valsT = [qkv.tile([P, S], bf16, name=f"valsT{c}")
                     for c in range(PAIRS)]

            def qk_proj(c):
                for (x_sb, wname, dst, bias_sb) in (
                    (xq_sb, "wq", qT[c], bq_sb if has_bq else None),
                    (xk_sb, "wk", kT[c], bk_sb if has_bk else None),
                ):
                    for s4 in range(NS4):
                        ps = psO.tile([P, SQ], f32, tag="psO")
                        for kc in range(KC):
                            nc.tensor.matmul(
                                ps[:],
                                lhsT=w_sb[wname][:, kc * EH + c * P:
                                                 kc * EH + (c + 1) * P],
                                rhs=x_sb[kc][:, s4 * SQ:(s4 + 1) * SQ],
                                start=(kc == 0),
                                stop=(kc == KC - 1 and bias_sb is None),
                            )
                        if bias_sb is not None:
                            nc.tensor.matmul(
                                ps[:],
                                lhsT=bias_sb[0:1, c * P:(c + 1) * P],
                                rhs=ones_sq[0:1, :],
                                start=False, stop=True,
                            )
                        nc.vector.tensor_copy(
                            out=dst[:, s4 * SQ:(s4 + 1) * SQ], in_=ps[:])

            # ---- attention for pair c ----
            def attention(c):
                for s4 in range(NS4):
                    sq0 = s4 * SQ
                    if causal:
                        sk_list = list(range(4 * (s4 + 1)))
                    else:
                        sk_list = list(range(NS16))
                    vals_ps = [psV.tile([D + 1, SQ], f32, tag="psV",
                                        name=f"vps{c}_{s4}_{h2}")
                               for h2 in range(2)]
                    n_sk = len(sk_list)
                    for i, sk in enumerate(sk_list):
                        sk0 = sk * P
                        first, last = (i == 0), (i == n_sk - 1)
                        e_t = [None, None]
                        for h2 in range(2):
                            sc = psS.tile([P, SQ], f32, tag="psS")
                            nc.tensor.matmul(
                                sc[:],
                                lhsT=kT[c][h2 * D:(h2 + 1) * D, sk0:sk0 + P],
                                rhs=qT[c][h2 * D:(h2 + 1) * D, sq0:sq0 + SQ],
                                start=True, stop=True,
                            )
                            e = expp.tile([P, SQ], bf16, tag="e")
                            nc.scalar.activation(e[:], sc[:], EXP, scale=0.125)
                            if causal:
                                jj = sk - 4 * s4
                                if jj >= 0:
                                    nc.vector.tensor_mul(
                                        e[:], e[:],
                                        cm_sb[:, jj * SQ:(jj + 1) * SQ])
                            else:
                                mt = mtp.tile([P, SQ], bf16, tag="mt")
                                nc.sync.dma_start(
                                    out=mt[:],
                                    in_=mt_d.ap()[sk0:sk0 + P, sq0:sq0 + SQ])
                                nc.vector.tensor_mul(e[:], e[:], mt[:])
                            e_t[h2] = e
                        for h2 in range(2):
                            hh = 2 * c + h2
                            nc.tensor.matmul(
                                vals_ps[h2][:],
                                lhsT=v_aug[sk][:, hh * (D + 1):
                                               (hh + 1) * (D + 1)],
                                rhs=e_t[h2][:],
                                start=first, stop=last,
                            )
                    for h2 in range(2):
                        rs = small.tile([1, SQ], f32, tag="rs")
                        nc.vector.reciprocal(rs[:], vals_ps[h2][D:D + 1, :])
                        bc = small.tile([D, SQ], f32, tag="bc")
                        nc.gpsimd.partition_broadcast(bc[:], rs[0:1, :])
                        nc.vector.tensor_mul(
                            valsT[c][h2 * D:(h2 + 1) * D, sq0:sq0 + SQ],
                            vals_ps[h2][0:D, :], bc[:])

            qk_proj(0)
            attention(0)
            qk_proj(1)
            attention(1)

            # ---- output projection ----
            out_view = out_d.ap()
            for s16 in range(NS16):
                for f2 in range(2):
                    ps = psO.tile([P, SQ], f32, tag="psO")
                    for ec in range(PAIRS):
                        nc.tensor.matmul(
                            ps[:],
                            lhsT=valsT[ec][:, s16 * P:(s16 + 1) * P],
                            rhs=wo_sb[:, ec * E + f2 * SQ:
                                      ec * E + (f2 + 1) * SQ],
                            start=(ec == 0), stop=(ec == PAIRS - 1),
                        )
                    ob = expp.tile([P, SQ], f32, tag="ob", name=f"ob{s16}_{f2}")
                    nc.vector.tensor_copy(out=ob[:], in_=ps[:])
                    nc.sync.dma_start(
                        out=out_view[s16 * P:(s16 + 1) * P,
                                     f2 * SQ:(f2 + 1) * SQ],
                        in_=ob[:],
                    )

    nc.compile()
    return nc


def _get_program(causal, has_bq, has_bk):
    key = (causal, has_bq, has_bk)
    if key not in _cache:
        _cache[key] = _build(*key)
    return _cache[key]


def _prepare(query, key, value, mask, Wq, bq, Wk, bk, Wv, bv, Wo, bo):
    query = np.asarray(query, dtype=np.float32)
    key = np.asarray(key, dtype=np.float32)
    value = np.asarray(value, dtype=np.float32)
    mask = np.asarray(mask)
    Wq, Wk, Wv, Wo = (np.asarray(w, dtype=np.float32) for w in (Wq, Wk, Wv, Wo))
    bq, bk, bv, bo = (np.asarray(b, dtype=np.float32) for b in (bq, bk, bv, bo))

    tril = np.tril(np.ones((S, S), mask.dtype))
    causal = all(np.array_equal(mask[b], tril) for b in range(B))
    has_bq = bool(np.any(bq))
    has_bk = bool(np.any(bk))
    nc = _get_program(causal, has_bq, has_bk)

    if causal:
        r = np.arange(P)[:, None]
        cidx = np.arange(SQ)[None, :]
        cm = np.concatenate(
            [(cidx >= r + P * jj).astype(BF16) for jj in range(4)], axis=0)

    in_maps = []
    for c in range(N_CORES):
        b, g = divmod(c, GPB)
        hs = g * EH
        m = {
            "xq": np.ascontiguousarray(query[b].T).astype(BF16),
            "xk": np.ascontiguousarray(key[b].T).astype(BF16),
            "xv": np.ascontiguousarray(value[b].T).astype(BF16),
            "wq": np.ascontiguousarray(Wq[hs:hs + EH, :].T).astype(BF16),
            "wk": np.ascontiguousarray(Wk[hs:hs + EH, :].T).astype(BF16),
            "wv": np.ascontiguousarray(Wv[hs:hs + EH, :].T).astype(BF16),
            "wo": np.ascontiguousarray(Wo[:, hs:hs + EH].T).astype(BF16),
        }
        if causal:
            m["cmask"] = cm
        else:
            m["maskT"] = np.ascontiguousarray(mask[b].T).astype(BF16)
        if has_bq:
            m["bq"] = bq[hs:hs + EH][None, :].astype(BF16)
        if has_bk:
            m["bk"] = bk[hs:hs + EH][None, :].astype(BF16)
        in_maps.append(m)

    shift = (Wo @ bv + bo).astype(np.float32)
    return nc, in_maps, shift


def _gather(results, shift):
    out = np.zeros((B, S, E), np.float32)
    for c in range(N_CORES):
        b = c // GPB
        out[b] += results[c]["outp"]
    out += shift[None, None, :]
    return out


def _profile_setup(inputs):
    nc, in_maps, _ = _prepare(**inputs)
    return in_maps, nc


def kernel(query, key, value, mask, Wq, bq, Wk, bk, Wv, bv, Wo, bo):
    from concourse.bass_utils import run_bass_kernel_spmd

    nc, in_maps, shift = _prepare(query, key, value, mask, Wq, bq, Wk, bk,
                                  Wv, bv, Wo, bo)
    res = run_bass_kernel_spmd(nc, in_maps, list(range(N_CORES)))
    return _gather(res.results, shift)


# revision 10
# speedup vs baseline: 1.3674x; 1.3674x over previous
"""Multi-head attention (B=2, S=2048, E=1024, H=16) on 8 TRN2 NeuronCores.

Sharding: core c -> batch b = c//4, head-group g = c%4 (4 heads each).
Megatron-style: col-sharded Wq/Wk/Wv, row-sharded Wo; host sums the 4
partial outputs per batch (the "all-reduce") and adds the bias terms.

Device kernel (per core), all bf16 matmuls with fp32 PSUM accumulate:
  qT/kT  [e=256, s] = W.T-slice @ x.T     (x passed pre-transposed)
  v_aug  [s, 4*65]  = x.T proj, + ones col per head (rowsum trick)
  scoresT[sk, sq]   = kT.T @ qT           (two heads packed in PE rows,
                                           both into one 2-bank PSUM tile)
  e = exp(scoresT/8)(*mask)  ->  vals_psum[65, sq] += v_aug.T @ e
  row 64 of vals_psum = softmax denominator; reciprocal (ACT) +
  partition_broadcast (GpSimd) + multiply (DVE)
  valsT  [e, s] = vals * 1/denominator ; partial_out = valsT.T @ WoT-slice
"""

import numpy as np
import ml_dtypes

B, S, E, H = 2, 2048, 1024, 16
D = E // H            # 64 head dim
N_CORES = 8
GPB = 4               # head-groups per batch
HC = H // GPB         # 4 heads per core
EH = HC * D           # 256 channels per core
PAIRS = HC // 2       # 2 head-pairs per core
P = 128               # partitions
SQ = 512              # sq tile (moving free dim)
KC = E // P           # 8 contraction chunks for projections
NS4 = S // SQ         # 4 sq chunks
NS16 = S // P         # 16

BF16 = ml_dtypes.bfloat16

_cache = {}


def _build(causal: bool, has_bq: bool, has_bk: bool):
    import concourse.bacc as bacc
    import concourse.mybir as mybir
    from concourse.tile import TileContext

    bf16 = mybir.dt.bfloat16
    f32 = mybir.dt.float32
    EXP = mybir.ActivationFunctionType.Exp

    nc = bacc.Bacc("TRN2", target_bir_lowering=False, debug=False,
                   num_devices=N_CORES)

    xq_d = nc.dram_tensor("xq", [E, S], bf16, kind="ExternalInput")
    xk_d = nc.dram_tensor("xk", [E, S], bf16, kind="ExternalInput")
    xv_d = nc.dram_tensor("xv", [E, S], bf16, kind="ExternalInput")
    wq_d = nc.dram_tensor("wq", [E, EH], bf16, kind="ExternalInput")
    wk_d = nc.dram_tensor("wk", [E, EH], bf16, kind="ExternalInput")
    wv_d = nc.dram_tensor("wv", [E, EH], bf16, kind="ExternalInput")
    wo_d = nc.dram_tensor("wo", [EH, E], bf16, kind="ExternalInput")
    if causal:
        cm_d = nc.dram_tensor("cmask", [4 * P, SQ], bf16, kind="ExternalInput")
    else:
        mt_d = nc.dram_tensor("maskT", [S, S], bf16, kind="ExternalInput")
    if has_bq:
        bq_d = nc.dram_tensor("bq", [1, EH], bf16, kind="ExternalInput")
    if has_bk:
        bk_d = nc.dram_tensor("bk", [1, EH], bf16, kind="ExternalInput")
    out_d = nc.dram_tensor("outp", [S, E], f32, kind="ExternalOutput")

    with TileContext(nc) as tc:
        with (
            tc.tile_pool(name="wpool", bufs=1) as wpool,
            tc.tile_pool(name="xq_pool", bufs=KC) as xq_pool,
            tc.tile_pool(name="xk_pool", bufs=KC) as xk_pool,
            tc.tile_pool(name="xv_pool", bufs=KC) as xv_pool,
            tc.tile_pool(name="qkv", bufs=1) as qkv,
            tc.tile_pool(name="vaug", bufs=NS16) as vaug_pool,
            tc.tile_pool(name="expp", bufs=6) as expp,
            tc.tile_pool(name="small", bufs=4) as small,
            tc.tile_pool(name="mtp", bufs=4) as mtp,
            tc.tile_pool(name="psS", bufs=2, space="PSUM") as psS,
            tc.tile_pool(name="psV", bufs=2, space="PSUM") as psV,
            tc.tile_pool(name="psO", bufs=2, space="PSUM") as psO,
        ):
            # ---- DMA loads, issued in need-order on two HWDGE queues ----
            # sync queue: wv, xv chunks (v-projection runs first)
            wv_sb = wpool.tile([P, KC * EH], bf16, name="wv_sb")
            nc.sync.dma_start(
                out=wv_sb[:].rearrange("p (kc e) -> p kc e", e=EH),
                in_=wv_d.ap().rearrange("(kc p) e -> p kc e", p=P),
            )
            xv_view = xv_d.ap().rearrange("(kc p) s -> kc p s", p=P)
            xv_sb = []
            for kc in range(KC):
                t = xv_pool.tile([P, S], bf16, name=f"xv{kc}", tag="xv")
                nc.sync.dma_start(out=t[:], in_=xv_view[kc])
                xv_sb.append(t)

            # scalar queue: wq, wk, xq, xk
            w_sb = {"wv": wv_sb}
            for nm, d in (("wq", wq_d), ("wk", wk_d)):
                t = wpool.tile([P, KC * EH], bf16, name=f"{nm}_sb")
                nc.scalar.dma_start(
                    out=t[:].rearrange("p (kc e) -> p kc e", e=EH),
                    in_=d.ap().rearrange("(kc p) e -> p kc e", p=P),
                )
                w_sb[nm] = t

            def load_x(pool, d, nm):
                view = d.ap().rearrange("(kc p) s -> kc p s", p=P)
                tiles = []
                for kc in range(KC):
                    t = pool.tile([P, S], bf16, name=f"{nm}{kc}", tag=nm)
                    nc.scalar.dma_start(out=t[:], in_=view[kc])
                    tiles.append(t)
                return tiles

            xq_sb = load_x(xq_pool, xq_d, "xq")
            xk_sb = load_x(xk_pool, xk_d, "xk")

            # sync queue: wo, cmask, biases
            wo_sb = wpool.tile([P, 2 * E], bf16, name="wo_sb")
            nc.sync.dma_start(
                out=wo_sb[:].rearrange("p (ec f) -> p ec f", f=E),
                in_=wo_d.ap().rearrange("(ec p) f -> p ec f", p=P),
            )
            if causal:
                cm_sb = wpool.tile([P, 4 * SQ], bf16, name="cm_sb")
                nc.sync.dma_start(
                    out=cm_sb[:].rearrange("p (j s) -> p j s", s=SQ),
                    in_=cm_d.ap().rearrange("(j p) s -> p j s", p=P),
                )
            if has_bq:
                bq_sb = wpool.tile([1, EH], bf16, name="bq_sb")
                nc.sync.dma_start(out=bq_sb[:], in_=bq_d.ap())
            if has_bk:
                bk_sb = wpool.tile([1, EH], bf16, name="bk_sb")
                nc.sync.dma_start(out=bk_sb[:], in_=bk_d.ap())
            if has_bq or has_bk:
                ones_sq = wpool.tile([1, SQ], bf16, name="ones_sq")
                nc.vector.memset(ones_sq[:], 1.0)

            # ---- V projection -> v_aug tiles [128, 4*65] ----
            v_aug = []
            for s16 in range(NS16):
                ps = psO.tile([P, EH], f32, tag="psO", name=f"psv{s16}")
                for kc in range(KC):
                    nc.tensor.matmul(
                        ps[:],
                        lhsT=xv_sb[kc][:, s16 * P:(s16 + 1) * P],
                        rhs=wv_sb[:, kc * EH:(kc + 1) * EH],
                        start=(kc == 0), stop=(kc == KC - 1),
                    )
                va = vaug_pool.tile([P, HC * (D + 1)], bf16,
                                    name=f"vaug{s16}", tag="vaug")
                nc.vector.tensor_copy(
                    out=va[:].rearrange("p (h z) -> p h z", z=D + 1)[:, :, 0:D],
                    in_=ps[:].rearrange("p (h d) -> p h d", d=D),
                )
                nc.vector.memset(
                    va[:].rearrange("p (h z) -> p h z", z=D + 1)[:, :, D:D + 1],
                    1.0,
                )
                v_aug.append(va)

            # per-(pair, s4chunk) tiles so consumers start per-chunk
            qT = [[qkv.tile([P, SQ], bf16, name=f"qT{c}_{s4}")
                   for s4 in range(NS4)] for c in range(PAIRS)]
            kT = [[qkv.tile([P, SQ], bf16, name=f"kT{c}_{s4}")
                   for s4 in range(NS4)] for c in range(PAIRS)]
            valsT = [[qkv.tile([P, SQ], bf16, name=f"valsT{c}_{s4}")
                      for s4 in range(NS4)] for c in range(PAIRS)]

            def qk_proj(c):
                for (x_sb, wname, dst, bias_sb) in (
                    (xq_sb, "wq", qT[c], bq_sb if has_bq else None),
                    (xk_sb, "wk", kT[c], bk_sb if has_bk else None),
                ):
                    for s4 in range(NS4):
                        ps = psO.tile([P, SQ], f32, tag="psO",
                                      name=f"psp{c}_{s4}")
                        for kc in range(KC):
                            nc.tensor.matmul(
                                ps[:],
                                lhsT=w_sb[wname][:, kc * EH + c * P:
                                                 kc * EH + (c + 1) * P],
                                rhs=x_sb[kc][:, s4 * SQ:(s4 + 1) * SQ],
                                start=(kc == 0),
                                stop=(kc == KC - 1 and bias_sb is None),
                            )
                        if bias_sb is not None:
                            nc.tensor.matmul(
                                ps[:],
                                lhsT=bias_sb[0:1, c * P:(c + 1) * P],
                                rhs=ones_sq[0:1, :],
                                start=False, stop=True,
                            )
                        nc.vector.tensor_copy(out=dst[s4][:], in_=ps[:])

            # ---- attention for pair c ----
            def attention(c):
                for s4 in range(NS4):
                    sq0 = s4 * SQ
                    if causal:
                        sk_list = list(range(4 * (s4 + 1)))
                    else:
                        sk_list = list(range(NS16))
                    vals_ps = [psV.tile([D + 1, SQ], f32, tag="psV",
                                        name=f"vps{c}_{s4}_{h2}")
                               for h2 in range(2)]
                    n_sk = len(sk_list)
                    for i, sk in enumerate(sk_list):
                        t4, col = sk // 4, (sk % 4) * P
                        first, last = (i == 0), (i == n_sk - 1)
                        sc = psS.tile([P, 2 * SQ], f32, tag="psS",
                                      name=f"sc{c}_{s4}_{sk}")
                        for h2 in range(2):
                            nc.tensor.matmul(
                                sc[:, h2 * SQ:(h2 + 1) * SQ],
                                lhsT=kT[c][t4][h2 * D:(h2 + 1) * D,
                                              col:col + P],
                                rhs=qT[c][s4][h2 * D:(h2 + 1) * D, :],
                                start=True, stop=True,
                            )
                        e = expp.tile([P, 2 * SQ], bf16, tag="e",
                                      name=f"e{c}_{s4}_{sk}")
                        nc.scalar.activation(e[:], sc[:], EXP, scale=0.125)
                        if causal:
                            jj = sk - 4 * s4
                            if jj >= 0:
                                cmv = cm_sb[:, jj * SQ:(jj + 1) * SQ]
                                nc.vector.tensor_mul(
                                    e[:].rearrange("p (u s) -> p u s", s=SQ),
                                    e[:].rearrange("p (u s) -> p u s", s=SQ),
                                    cmv.rearrange("p (u s) -> p u s", s=SQ
                                                  ).broadcast_to([P, 2, SQ]),
                                )
                        else:
                            mt = mtp.tile([P, SQ], bf16, tag="mt",
                                          name=f"mt{c}_{s4}_{sk}")
                            nc.sync.dma_start(
                                out=mt[:],
                                in_=mt_d.ap()[sk * P:(sk + 1) * P,
                                              sq0:sq0 + SQ])
                            nc.vector.tensor_mul(
                                e[:].rearrange("p (u s) -> p u s", s=SQ),
                                e[:].rearrange("p (u s) -> p u s", s=SQ),
                                mt[:].rearrange("p (u s) -> p u s", s=SQ
                                                ).broadcast_to([P, 2, SQ]),
                            )
                        for h2 in range(2):
                            hh = 2 * c + h2
                            nc.tensor.matmul(
                                vals_ps[h2][:],
                                lhsT=v_aug[sk][:, hh * (D + 1):
                                               (hh + 1) * (D + 1)],
                                rhs=e[:, h2 * SQ:(h2 + 1) * SQ],
                                start=first, stop=last,
                            )
                    for h2 in range(2):
                        rsum = small.tile([1, SQ], f32, tag="rsum",
                                          name=f"rsum{c}_{s4}_{h2}")
                        nc.vector.tensor_copy(
                            out=rsum[:], in_=vals_ps[h2][D:D + 1, :])
                        rs = small.tile([1, SQ], f32, tag="rs",
                                        name=f"rs{c}_{s4}_{h2}")
                        nc.vector.reciprocal_approx_fast(
                            out=rs[:], in_=rsum[:])
                        bc = small.tile([D, SQ], f32, tag="bc",
                                        name=f"bc{c}_{s4}_{h2}")
                        nc.gpsimd.partition_broadcast(bc[:], rs[0:1, :])
                        nc.vector.tensor_mul(
                            valsT[c][s4][h2 * D:(h2 + 1) * D, :],
                            vals_ps[h2][0:D, :], bc[:])

            qk_proj(0)
            attention(0)
            qk_proj(1)
            attention(1)

            # ---- output projection ----
            out_view = out_d.ap()
            for s16 in range(NS16):
                s4, pcol = s16 // 4, (s16 % 4) * P
                for f2 in range(2):
                    ps = psO.tile([P, SQ], f32, tag="psO",
                                  name=f"pso{s16}_{f2}")
                    for ec in range(PAIRS):
                        nc.tensor.matmul(
                            ps[:],
                            lhsT=valsT[ec][s4][:, pcol:pcol + P],
                            rhs=wo_sb[:, ec * E + f2 * SQ:
                                      ec * E + (f2 + 1) * SQ],
                            start=(ec == 0), stop=(ec == PAIRS - 1),
                        )
                    ob = expp.tile([P, SQ], f32, tag="ob",
                                   name=f"ob{s16}_{f2}")
                    nc.vector.tensor_copy(out=ob[:], in_=ps[:])
                    nc.sync.dma_start(
                        out=out_view[s16 * P:(s16 + 1) * P,
                                     f2 * SQ:(f2 + 1) * SQ],
                        in_=ob[:],
                    )

    nc.compile()
    return nc


def _get_program(causal, has_bq, has_bk):
    key = (causal, has_bq, has_bk)
    if key not in _cache:
        _cache[key] = _build(*key)
    return _cache[key]


def _prepare(query, key, value, mask, Wq, bq, Wk, bk, Wv, bv, Wo, bo):
    query = np.asarray(query, dtype=np.float32)
    key = np.asarray(key, dtype=np.float32)
    value = np.asarray(value, dtype=np.float32)
    mask = np.asarray(mask)
    Wq, Wk, Wv, Wo = (np.asarray(w, dtype=np.float32) for w in (Wq, Wk, Wv, Wo))
    bq, bk, bv, bo = (np.asarray(b, dtype=np.float32) for b in (bq, bk, bv, bo))

    tril = np.tril(np.ones((S, S), mask.dtype))
    causal = all(np.array_equal(mask[b], tril) for b in range(B))
    has_bq = bool(np.any(bq))
    has_bk = bool(np.any(bk))
    nc = _get_program(causal, has_bq, has_bk)

    if causal:
        r = np.arange(P)[:, None]
        cidx = np.arange(SQ)[None, :]
        cm = np.concatenate(
            [(cidx >= r + P * jj).astype(BF16) for jj in range(4)], axis=0)

    in_maps = []
    for c in range(N_CORES):
        b, g = divmod(c, GPB)
        hs = g * EH
        m = {
            "xq": np.ascontiguousarray(query[b].T).astype(BF16),
            "xk": np.ascontiguousarray(key[b].T).astype(BF16),
            "xv": np.ascontiguousarray(value[b].T).astype(BF16),
            "wq": np.ascontiguousarray(Wq[hs:hs + EH, :].T).astype(BF16),
            "wk": np.ascontiguousarray(Wk[hs:hs + EH, :].T).astype(BF16),
            "wv": np.ascontiguousarray(Wv[hs:hs + EH, :].T).astype(BF16),
            "wo": np.ascontiguousarray(Wo[:, hs:hs + EH].T).astype(BF16),
        }
        if causal:
            m["cmask"] = cm
        else:
            m["maskT"] = np.ascontiguousarray(mask[b].T).astype(BF16)
        if has_bq:
            m["bq"] = bq[hs:hs + EH][None, :].astype(BF16)
        if has_bk:
            m["bk"] = bk[hs:hs + EH][None, :].astype(BF16)
        in_maps.append(m)

    shift = (Wo @ bv + bo).astype(np.float32)
    return nc, in_maps, shift


def _gather(results, shift):
    out = np.zeros((B, S, E), np.float32)
    for c in range(N_CORES):
        b = c // GPB
        out[b] += results[c]["outp"]
    out += shift[None, None, :]
    return out


def _profile_setup(inputs):
    nc, in_maps, _ = _prepare(**inputs)
    return in_maps, nc


def kernel(query, key, value, mask, Wq, bq, Wk, bk, Wv, bv, Wo, bo):
    from concourse.bass_utils import run_bass_kernel_spmd

    nc, in_maps, shift = _prepare(query, key, value, mask, Wq, bq, Wk, bk,
                                  Wv, bv, Wo, bo)
    res = run_bass_kernel_spmd(nc, in_maps, list(range(N_CORES)))
    return _gather(res.results, shift)
